# revision 7
# baseline (speedup 1.0000x reference)
"""Trainium2 Bass kernel for nn_BCE_Loss (focal-style BCE-with-logits, mean).

Reference math per anchor row x[0:3] (logits) and integer target c:
    col = 0 if c==1 else 1 if c==3 else 2
    t   = one_hot(col, 3)
    w   = (1-pt)^2,  pt = x*t + (1-x)*(1-t)        [from detached logits]
    bce = max(x,0) - x*t + log1p(exp(-|x|))
    out = mean(w * bce)

Per element this is (x-t)^2 * softplus(v), v = x*(1-2t).  With
g = 0.5 - t in {+-0.5} and h = g*x, two identities remove all per-element
weight math:
    v         = 2*h
    (x - t)^2 = x^2 + 2*h - x - g + 0.5
so the loss sum becomes four dot-products against sp = softplus(v) plus a
plain sum of sp:
    S = sum x^2*sp + 2 sum h*sp - sum x*sp - sum g*sp + 0.5 sum sp

Engine split (per 128x3072 tile, 8 tiles/core):
    DVE     three contiguous one-hot compares from int8 targ (tensor_scalar
            single-src 2x_2p), then h = g*x and xsq = x*x (bf16 2x_1p)
    GPSIMD  three strided interleave-copies of the compare results into the
            [t,3]-interleaved g tile (1-input Q7 ops run near line rate and
            the 6-byte write stride stays under the 8B cliff; walrus rejects
            tensor_scalar on Pool but tensor_copy lowers fine)
    ACT     E = Exp(2h) ; sp = Ln(E + 1) with accum_out giving the
            per-partition sum of sp for free (the 0.5 term)
    PE      per 128-chunk: stationary sp_c, moving [xsq|h|x|g] chunk slices
            (FD=512 via a 4-slot mega-tile view) accumulating the four diag
            sums in one PSUM [128,512] bank across all chunks and tiles
x is cast f32->bf16 by the SWDGE cast-DMA straight into the mega-tile's x
slot; targ is shipped as int8 (values 0..4) so the per-core DMA floor is
~38us instead of ~47us.

Sharding: pure data-parallel across 8 NeuronCores - each core takes a
contiguous block of anchors; per-core output is a single partial sum; the
host sums the 8 partials and divides by the element count.
"""

import numpy as np

import concourse.bacc as bacc
import concourse.bass as bass
import concourse.mybir as mybir
from concourse import bass_utils
from concourse.alu_op_type import AluOpType
from concourse.tile import TileContext

N_CORES = 8
N_ANCHORS = 8388608
N_CLASSES = 3
N_SHARD = N_ANCHORS // N_CORES  # 1048576
P = 128  # SBUF partitions
T = 1024  # anchor rows per partition per tile
NT = N_SHARD // (P * T)  # 8 tiles per core
F = N_CLASSES * T  # free dim of an x tile
MM = 128  # diag-trick matmul chunk width
N_CHUNK = F // MM
NG = 4  # PE moving groups: [xsq, h, x, g]


class _Bacc(bacc.Bacc):
    """Bacc with the ACT table pinned to natural_log_exp_and_others.

    The default chooser puts Exp in exp_and_others and Ln in natural_log,
    reloading tables every tile (~2.7us each). Both live in
    natural_log_exp_and_others; emptying every other set (positions kept -
    act_func_set_id is the index into act_info.json) forces one load."""

    _ACT_SET = "natural_log_exp_and_others"

    def insert_act_table_loads(self):
        import bass_rust as _bass_rust

        from concourse.hw_specs import get_activation_tables

        has_activation = any(
            isinstance(i, mybir.InstActivation)
            for b in self.main_func.blocks
            for i in b.instructions
        )
        if not has_activation:
            return
        tables = [
            (name, (fns if name == self._ACT_SET else set()))
            for name, fns in get_activation_tables(self.m.arch).items()
        ]
        _bass_rust.insert_act_table_loads(self, tables)


def _build_nc() -> bass.Bass:
    nc = _Bacc("TRN2", target_bir_lowering=False, num_swdge_queues=4)
    pred = nc.dram_tensor(
        "pred", [N_SHARD, N_CLASSES], mybir.dt.float32, kind="ExternalInput"
    )
    targ = nc.dram_tensor("targ8", [N_SHARD], mybir.dt.int8, kind="ExternalInput")
    msgn = nc.dram_tensor("msgn", [P, NG * MM], mybir.dt.bfloat16, kind="ExternalInput")
    out = nc.dram_tensor("out", [1], mybir.dt.float32, kind="ExternalOutput")

    xv = pred.rearrange("(n p t) m -> n p (t m)", p=P, t=T)
    tv = targ.rearrange("(n p t) -> n p t", p=P, t=T)

    with TileContext(nc) as tc:
        with (
            tc.tile_pool(name="io", bufs=4) as io,
            tc.tile_pool(name="cc", bufs=3) as ccp,
            tc.tile_pool(name="mega", bufs=3) as megap,
            tc.tile_pool(name="ep", bufs=2) as ep,
            tc.tile_pool(name="spp", bufs=3) as spp,
            tc.tile_pool(name="singles", bufs=1) as singles,
            tc.tile_pool(name="psum", bufs=1, space="PSUM") as psum,
        ):
            ones_f = singles.tile([P, 1], mybir.dt.float32)
            nc.vector.memset(ones_f, 1.0)
            accsp = singles.tile([P, NT], mybir.dt.float32)
            psA = psum.tile([P, NG * MM], mybir.dt.float32)

            for i in range(NT):
                # mega-tile slots: 0 = xsq, 1 = h, 2 = x, 3 = g
                B = megap.tile([P, NG * F], mybir.dt.bfloat16)
                B3 = B.rearrange("p (s f) -> p s f", s=NG)
                xb = B3[:, 2, :]
                g = B3[:, 3, :]
                g3 = g.rearrange("p (t m) -> p t m", m=N_CLASSES)

                # x loaded with f32->bf16 cast in the DMA datapath (SWDGE)
                tg = io.tile([P, T], mybir.dt.int8)
                nc.gpsimd.dma_start(out=xb, in_=xv[i])
                nc.sync.dma_start(out=tg, in_=tv[i])

                # contiguous one-hot compares on DVE (single-src ts -> 2x_2p)
                c0 = ccp.tile([P, T], mybir.dt.bfloat16)
                c1 = ccp.tile([P, T], mybir.dt.bfloat16)
                c2 = ccp.tile([P, T], mybir.dt.bfloat16)
                nc.vector.tensor_scalar(
                    out=c0, in0=tg, scalar1=1, scalar2=0.5,
                    op0=AluOpType.not_equal, op1=AluOpType.subtract)
                nc.vector.tensor_scalar(
                    out=c1, in0=tg, scalar1=3, scalar2=0.5,
                    op0=AluOpType.not_equal, op1=AluOpType.subtract)
                # c2 = 0.5 - c0 - c1  (reverse0: scalar - in0)
                ic2 = nc.vector.scalar_tensor_tensor(
                    out=c2, in0=c0, scalar=0.5, in1=c1,
                    op0=AluOpType.subtract, op1=AluOpType.subtract)
                ic2.ins.reverse0 = True
                # strided interleave into g on the otherwise-idle GPSIMD
                nc.gpsimd.tensor_copy(out=g3[:, :, 0], in_=c0)
                nc.gpsimd.tensor_copy(out=g3[:, :, 1], in_=c1)
                nc.gpsimd.tensor_copy(out=g3[:, :, 2], in_=c2)

                # h = g*x ; xsq = x*x
                nc.vector.tensor_tensor(
                    out=B3[:, 1, :], in0=g, in1=xb, op=AluOpType.mult)
                nc.vector.tensor_tensor(
                    out=B3[:, 0, :], in0=xb, in1=xb, op=AluOpType.mult)

                # E = exp(2h) = e^v ; sp = ln(E + 1) = softplus(v)
                E = ep.tile([P, F], mybir.dt.bfloat16)
                nc.scalar.activation(
                    out=E, in_=B3[:, 1, :],
                    func=mybir.ActivationFunctionType.Exp, scale=2.0)
                sp = spp.tile([P, F], mybir.dt.bfloat16)
                nc.scalar.activation(
                    out=sp, in_=E, func=mybir.ActivationFunctionType.Ln,
                    bias=1.0, accum_out=accsp[:, i : i + 1])

                # PE: psA += sp_c^T @ [xsq_c | h_c | x_c | g_c]; the four
                # 128-col group diagonals accumulate the four dot products
                for c in range(N_CHUNK):
                    s = slice(c * MM, (c + 1) * MM)
                    nc.tensor.matmul(
                        psA[:, :], sp[:, s], B3[:, :, s],
                        start=(i == 0 and c == 0),
                        stop=(i == NT - 1 and c == N_CHUNK - 1))

            # epilogue: S = (+1,+2,-1,-1) . group diags + 0.5 * sum accsp
            msgn_t = singles.tile([P, NG * MM], mybir.dt.bfloat16)
            nc.sync.dma_start(out=msgn_t, in_=msgn[:, :])
            dm = singles.tile([P, NG * MM], mybir.dt.float32)
            nc.vector.tensor_tensor(out=dm, in0=psA, in1=msgn_t, op=AluOpType.mult)
            r1 = singles.tile([P, 1], mybir.dt.float32)
            nc.vector.tensor_reduce(
                out=r1, in_=dm, axis=mybir.AxisListType.X, op=AluOpType.add)
            racc = singles.tile([P, 1], mybir.dt.float32)
            nc.vector.tensor_reduce(
                out=racc, in_=accsp, axis=mybir.AxisListType.X, op=AluOpType.add)
            tot = singles.tile([P, 1], mybir.dt.float32)
            nc.vector.scalar_tensor_tensor(
                out=tot, in0=racc, scalar=0.5, in1=r1,
                op0=AluOpType.mult, op1=AluOpType.add)

            psT = psum.tile([1, 1], mybir.dt.float32)
            nc.tensor.matmul(psT[:, :], ones_f[:, :], tot[:, :], start=True, stop=True)
            res = singles.tile([1, 1], mybir.dt.float32)
            nc.vector.tensor_copy(out=res, in_=psT)
            nc.sync.dma_start(out=out[:], in_=res[0, :])

    nc.compile()
    return nc


_cache: dict[str, bass.Bass] = {}
last_results = None  # BassKernelResults of the most recent run (for test.py)


def _get_nc() -> bass.Bass:
    if "nc" not in _cache:
        _cache["nc"] = _build_nc()
    return _cache["nc"]


def _msgn_bf16() -> np.ndarray:
    import ml_dtypes

    coefs = [1.0, 2.0, -1.0, -1.0]  # xsq, h, x, g
    m = np.zeros((P, NG * MM), dtype=np.float32)
    idx = np.arange(P)
    for s, cf in enumerate(coefs):
        m[idx, s * MM + idx] = cf
    return m.astype(ml_dtypes.bfloat16)


def kernel(pred: np.ndarray, targ: np.ndarray, *, trace: bool = False) -> np.ndarray:
    global last_results
    pred = np.ascontiguousarray(np.asarray(pred, dtype=np.float32))
    targ = np.asarray(targ)
    assert pred.shape == (N_ANCHORS, N_CLASSES), pred.shape
    assert targ.shape == (N_ANCHORS,), targ.shape

    # lossless downcast of the index tensor (values 0..4) for the device
    targ8 = np.ascontiguousarray(targ.astype(np.int8))

    nc = _get_nc()
    msgn = _msgn_bf16()

    in_maps = []
    for c in range(N_CORES):
        in_maps.append({
            "pred": pred[c * N_SHARD : (c + 1) * N_SHARD],
            "targ8": targ8[c * N_SHARD : (c + 1) * N_SHARD],
            "msgn": msgn,
        })

    res = bass_utils.run_bass_kernel_spmd(
        nc, in_maps, core_ids=list(range(N_CORES)), trace=trace
    )
    last_results = res

    total = np.float64(0.0)
    for r in res.results:
        total += np.float64(r["out"][0])
    mean = total / (N_ANCHORS * N_CLASSES)
    return np.float32(mean)


# revision 13
# speedup vs baseline: 1.7126x; 1.7126x over previous
"""Trainium2 Bass kernel for nn_BCE_Loss (focal-style BCE-with-logits, mean).

Reference math per anchor row x[0:3] (logits) and integer target c:
    col = 0 if c==1 else 1 if c==3 else 2
    t   = one_hot(col, 3)
    w   = (1-pt)^2,  pt = x*t + (1-x)*(1-t)        [from detached logits]
    bce = max(x,0) - x*t + log1p(exp(-|x|))
    out = mean(w * bce)

Per element this is (x-t)^2 * softplus(v), v = x*(1-2t).  With
g = 0.5 - t in {+-0.5} and h = g*x, two identities remove all per-element
weight math:
    v         = 2*h
    (x - t)^2 = x^2 + 2*h - (x - 0.5) - g       [the -0.5 absorbs the +0.5]
so the loss sum becomes four dot products against sp = softplus(v):
    S = sum x^2*sp + 2 sum h*sp - sum (x-0.5)*sp - sum g*sp

Layout: the host ships pred CLASS-PLANAR (per-core [3, n] slab, a pure
relayout of the sharded block) so every on-device tensor is contiguous
blocked [x0|x1|x2] instead of [t,3]-interleaved - elementwise ops do not
care about element order and the mask build loses its 1x-mode strided
writes (measured 3x cost) entirely.  targ ships as bf16 (values 0..4 are
exact) so the one-hot compares run in the DVE's 4x mode.

Engine split (per 128x3072 tile, 8 tiles/core):
    DVE   g0/g1 compares, q = |targ-2|, g2 = (q==1)-0.5 (all 4x ~330ns),
          xp = x-0.5 (4x), h = g*x and xsq = x*x (bf16 2x_1p)
    ACT   E = Exp(2h) ; sp = Ln(E + 1)   (the only table work; one
          table-set load via the pinned natural_log_exp_and_others set)
    PE    per 128-chunk: stationary sp_c, moving [xsq|h|xp|g] chunk slices
          (FD=512 via a 4-slot mega-tile view, streams at ~216ns/chunk)
          accumulating the four diag sums in one PSUM [128,512] bank
    x is cast f32->bf16 by the SWDGE cast-DMA straight into its io tile.

Sharding: pure data-parallel across 8 NeuronCores - each core takes a
contiguous block of anchors; per-core output is a single partial sum; the
host sums the 8 partials and divides by the element count.
"""

import numpy as np

import concourse.bacc as bacc
import concourse.bass as bass
import concourse.mybir as mybir
from concourse import bass_utils
from concourse.alu_op_type import AluOpType
from concourse.tile import TileContext

N_CORES = 8
N_ANCHORS = 8388608
N_CLASSES = 3
N_SHARD = N_ANCHORS // N_CORES  # 1048576
P = 128  # SBUF partitions
T = 1024  # anchor rows per partition per tile
NT = N_SHARD // (P * T)  # 8 tiles per core
F = N_CLASSES * T  # free dim of an x tile
MM = 128  # diag-trick matmul chunk width
N_CHUNK = F // MM
NG = 4  # PE moving groups: [xsq, h, xp, g]


class _Bacc(bacc.Bacc):
    """Bacc with the ACT table pinned to natural_log_exp_and_others.

    The default chooser puts Exp in exp_and_others and Ln in natural_log,
    reloading tables every tile (~2.7us each). Both live in
    natural_log_exp_and_others; emptying every other set (positions kept -
    act_func_set_id is the index into act_info.json) forces one load."""

    _ACT_SET = "natural_log_exp_and_others"

    def insert_act_table_loads(self):
        import bass_rust as _bass_rust

        from concourse.hw_specs import get_activation_tables

        has_activation = any(
            isinstance(i, mybir.InstActivation)
            for b in self.main_func.blocks
            for i in b.instructions
        )
        if not has_activation:
            return
        tables = [
            (name, (fns if name == self._ACT_SET else set()))
            for name, fns in get_activation_tables(self.m.arch).items()
        ]
        _bass_rust.insert_act_table_loads(self, tables)


def _build_nc() -> bass.Bass:
    nc = _Bacc("TRN2", target_bir_lowering=False, num_swdge_queues=4)
    predt = nc.dram_tensor(
        "predt", [N_CLASSES * N_SHARD], mybir.dt.float32, kind="ExternalInput"
    )
    targ = nc.dram_tensor("targb", [N_SHARD], mybir.dt.bfloat16, kind="ExternalInput")
    msgn = nc.dram_tensor("msgn", [P, NG * MM], mybir.dt.bfloat16, kind="ExternalInput")
    out = nc.dram_tensor("out", [1], mybir.dt.float32, kind="ExternalOutput")

    # class-planar: element (j, n, p, t) -> tile n, partition p, col j*T+t
    xv = predt.rearrange("(j n p t) -> n p j t", j=N_CLASSES, p=P, t=T)
    tv = targ.rearrange("(n p t) -> n p t", p=P, t=T)

    with TileContext(nc) as tc:
        with (
            tc.tile_pool(name="io", bufs=3) as io,
            tc.tile_pool(name="mega", bufs=3) as megap,
            tc.tile_pool(name="ep", bufs=2) as ep,
            tc.tile_pool(name="spp", bufs=3) as spp,
            tc.tile_pool(name="singles", bufs=1) as singles,
            tc.tile_pool(name="psum", bufs=1, space="PSUM") as psum,
        ):
            ones_f = singles.tile([P, 1], mybir.dt.float32)
            nc.vector.memset(ones_f, 1.0)
            psA = psum.tile([P, NG * MM], mybir.dt.float32)

            for i in range(NT):
                # mega-tile slots: 0 = xsq, 1 = h, 2 = xp = x-0.5, 3 = g
                B = megap.tile([P, NG * F], mybir.dt.bfloat16)
                B3 = B.rearrange("p (s f) -> p s f", s=NG)
                g = B3[:, 3, :]
                gj = g.rearrange("p (j t) -> p j t", j=N_CLASSES)

                # x loaded with f32->bf16 cast in the DMA datapath (SWDGE)
                xr = io.tile([P, F], mybir.dt.bfloat16)
                xr3 = xr.rearrange("p (j t) -> p j t", j=N_CLASSES)
                tg = io.tile([P, T], mybir.dt.bfloat16)
                nc.gpsimd.dma_start(out=xr3, in_=xv[i])
                nc.sync.dma_start(out=tg, in_=tv[i])

                # one-hot g planes, all contiguous writes:
                # g0 = (targ != 1) - 0.5 ; g1 = (targ != 3) - 0.5
                nc.vector.tensor_scalar(
                    out=gj[:, 0, :], in0=tg, scalar1=1, scalar2=0.5,
                    op0=AluOpType.not_equal, op1=AluOpType.subtract)
                nc.vector.tensor_scalar(
                    out=gj[:, 1, :], in0=tg, scalar1=3, scalar2=0.5,
                    op0=AluOpType.not_equal, op1=AluOpType.subtract)
                # g2 = 0.5 - g0 - g1  (reverse0: scalar - in0)
                ic2 = nc.vector.scalar_tensor_tensor(
                    out=gj[:, 2, :], in0=gj[:, 0, :], scalar=0.5,
                    in1=gj[:, 1, :],
                    op0=AluOpType.subtract, op1=AluOpType.subtract)
                ic2.ins.reverse0 = True

                # xp = x - 0.5 ; h = g*x ; xsq = x*x
                nc.vector.tensor_scalar(
                    out=B3[:, 2, :], in0=xr, scalar1=0.5, scalar2=0,
                    op0=AluOpType.subtract, op1=AluOpType.add)
                nc.vector.tensor_tensor(
                    out=B3[:, 1, :], in0=g, in1=xr, op=AluOpType.mult)
                nc.vector.tensor_tensor(
                    out=B3[:, 0, :], in0=xr, in1=xr, op=AluOpType.mult)

                # E = exp(2h) = e^v ; sp = ln(E + 1) = softplus(v)
                E = ep.tile([P, F], mybir.dt.bfloat16)
                nc.scalar.activation(
                    out=E, in_=B3[:, 1, :],
                    func=mybir.ActivationFunctionType.Exp, scale=2.0)
                sp = spp.tile([P, F], mybir.dt.bfloat16)
                nc.scalar.activation(
                    out=sp, in_=E, func=mybir.ActivationFunctionType.Ln,
                    bias=1.0)

                # PE: psA += sp_c^T @ [xsq_c | h_c | xp_c | g_c]; the four
                # 128-col group diagonals accumulate the four dot products
                for c in range(N_CHUNK):
                    s = slice(c * MM, (c + 1) * MM)
                    nc.tensor.matmul(
                        psA[:, :], sp[:, s], B3[:, :, s],
                        start=(i == 0 and c == 0),
                        stop=(i == NT - 1 and c == N_CHUNK - 1))

            # epilogue: S = (+1,+2,-1,-1) . group diags
            msgn_t = singles.tile([P, NG * MM], mybir.dt.bfloat16)
            nc.sync.dma_start(out=msgn_t, in_=msgn[:, :])
            dm = singles.tile([P, NG * MM], mybir.dt.float32)
            nc.vector.tensor_tensor(out=dm, in0=psA, in1=msgn_t, op=AluOpType.mult)
            r1 = singles.tile([P, 1], mybir.dt.float32)
            nc.vector.tensor_reduce(
                out=r1, in_=dm, axis=mybir.AxisListType.X, op=AluOpType.add)

            psT = psum.tile([1, 1], mybir.dt.float32)
            nc.tensor.matmul(psT[:, :], ones_f[:, :], r1[:, :], start=True, stop=True)
            res = singles.tile([1, 1], mybir.dt.float32)
            nc.vector.tensor_copy(out=res, in_=psT)
            nc.sync.dma_start(out=out[:], in_=res[0, :])

    nc.compile()
    return nc


_cache: dict[str, bass.Bass] = {}
last_results = None  # BassKernelResults of the most recent run (for test.py)


def _get_nc() -> bass.Bass:
    if "nc" not in _cache:
        _cache["nc"] = _build_nc()
    return _cache["nc"]


def _msgn_bf16() -> np.ndarray:
    import ml_dtypes

    coefs = [1.0, 2.0, -1.0, -1.0]  # xsq, h, xp, g
    m = np.zeros((P, NG * MM), dtype=np.float32)
    idx = np.arange(P)
    for s, cf in enumerate(coefs):
        m[idx, s * MM + idx] = cf
    return m.astype(ml_dtypes.bfloat16)


def kernel(pred: np.ndarray, targ: np.ndarray, *, trace: bool = False) -> np.ndarray:
    global last_results
    import ml_dtypes

    pred = np.ascontiguousarray(np.asarray(pred, dtype=np.float32))
    targ = np.asarray(targ)
    assert pred.shape == (N_ANCHORS, N_CLASSES), pred.shape
    assert targ.shape == (N_ANCHORS,), targ.shape

    # lossless bf16 image of the index tensor (values 0..4 are exact)
    targb = np.ascontiguousarray(targ.astype(ml_dtypes.bfloat16))

    nc = _get_nc()
    msgn = _msgn_bf16()

    in_maps = []
    for c in range(N_CORES):
        sl = slice(c * N_SHARD, (c + 1) * N_SHARD)
        # per-core class-planar relayout of the sharded block
        predt = np.ascontiguousarray(pred[sl].T).reshape(-1)
        in_maps.append({
            "predt": predt,
            "targb": targb[sl],
            "msgn": msgn,
        })

    res = bass_utils.run_bass_kernel_spmd(
        nc, in_maps, core_ids=list(range(N_CORES)), trace=trace
    )
    last_results = res

    total = np.float64(0.0)
    for r in res.results:
        total += np.float64(r["out"][0])
    mean = total / (N_ANCHORS * N_CLASSES)
    return np.float32(mean)
